# revision 5
# baseline (speedup 1.0000x reference)
"""Ensemble of 100 independent 3-layer MLPs on 8 Trainium2 NeuronCores.

Reference computation (E=100, D=2048, H1=512, H2=256, O=1, B=1024):
    h1  = relu(x @ W1[e] + b1[e])      [B, H1]  per expert
    h2  = relu(h1 @ W2[e] + b2[e])     [B, H2]
    out = h2 @ W3[e] + b3[e]           [B, 1]
    result[b, 0, e] = out[b]           -> [B, 1, E]

Sharding: expert-parallel. E=100 padded to 104 = 8 cores x 13 experts.
Each core gets its 13 experts' weights plus a replicated transposed
input xT; it computes out[e_local, b] and the host concatenates.

On-chip layout: activations are kept feature-major ([feature, batch]),
so every layer is matmul(psum, lhsT=W_tile[K,M], rhs=act[K,N]) with the
contraction on partitions and the output already transposed for the
next layer. The only real transpose (x -> xT) happens on the host.

All matmul operands are float32r: fp32 storage, single-pass FP22
matmul on the PE (same throughput as bf16, ~1e-4 relative error).
Layer 3 (O=1) is folded into a block-diagonal [128, 16] lhsT per
expert so each expert accumulates into its own PSUM partition row of a
single [16, 512] accumulator pair held across the whole kernel.
"""

import sys

if "/opt/trn_rl_repo" not in sys.path:
    sys.path.insert(0, "/opt/trn_rl_repo")

import numpy as np

import concourse.bass as bass
import concourse.tile as tile
from concourse import bacc, mybir
from concourse.bass import ts
from concourse.bass_utils import run_bass_kernel_spmd

F32 = mybir.dt.float32
F32R = mybir.dt.float32r
RELU = mybir.ActivationFunctionType.Relu
IDENT = mybir.ActivationFunctionType.Identity

E, D, H1, H2, B = 100, 2048, 512, 256, 1024
NCORES = 8
EPC = 13          # experts per core (104 padded)
P = 128
KD = D // P       # 16 k-tiles for layer 1
K1 = H1 // P      # 4 k-tiles for layer 2
K2 = H2 // P      # 2 k-tiles for layer 3
M1 = H1 // P      # 4 m-tiles layer 1
M2 = H2 // P      # 2 m-tiles layer 2
NB = 2            # batch split: 2 x 512
NF = B // NB      # 512

_CACHE = {}


def _build():
    nc = bacc.Bacc("TRN2", target_bir_lowering=False)

    xtd = nc.dram_tensor("xt", [D, B], F32R, kind="ExternalInput")
    w1d = nc.dram_tensor("w1", [EPC, D, H1], F32R, kind="ExternalInput")
    w2d = nc.dram_tensor("w2", [EPC, H1, H2], F32R, kind="ExternalInput")
    w3d = nc.dram_tensor("w3", [EPC, K2, P, 16], F32R, kind="ExternalInput")
    biasd = nc.dram_tensor("bias", [P, 80], F32, kind="ExternalInput")
    outd = nc.dram_tensor("out", [16, B], F32, kind="ExternalOutput")

    with tile.TileContext(nc) as tc:
        with (
            tc.tile_pool(name="const", bufs=1) as cpool,
            tc.tile_pool(name="w1p", bufs=2) as w1pool,
            tc.tile_pool(name="w2p", bufs=1) as w2pool,
            tc.tile_pool(name="h1p", bufs=2) as h1pool,
            tc.tile_pool(name="h2p", bufs=2) as h2pool,
            tc.tile_pool(name="ps1p", bufs=3, space="PSUM") as ps1pool,
            tc.tile_pool(name="ps2p", bufs=2, space="PSUM") as ps2pool,
            tc.tile_pool(name="ps3p", bufs=2, space="PSUM") as ps3pool,
        ):
            xt = cpool.tile([P, KD, B], F32R)
            for k in range(KD):
                nc.sync.dma_start(xt[:, k, :], xtd[ts(k, P), :])
            w3t = cpool.tile([P, EPC, K2, 16], F32R)
            nc.sync.dma_start(
                w3t[:], w3d[:].rearrange("e k p j -> p e k j")
            )
            biast = cpool.tile([P, 80], F32)
            nc.sync.dma_start(biast[:], biasd[:])
            outt = cpool.tile([16, B], F32)
            nc.gpsimd.memset(outt[:], 0.0)

            for e in range(EPC):
                w1t = w1pool.tile([P, KD, H1], F32R)
                nc.sync.dma_start(
                    w1t[:], w1d[e].rearrange("(k p) h -> p k h", p=P)
                )
                w2t = w2pool.tile([P, K1, H2], F32R)
                nc.sync.dma_start(
                    w2t[:], w2d[e].rearrange("(k p) h -> p k h", p=P)
                )
                for n in range(NB):
                    h1t = h1pool.tile([P, K1, NF], F32R)
                    for m in range(M1):
                        ps = ps1pool.tile([P, NF], F32)
                        for k in range(KD):
                            nc.tensor.matmul(
                                ps[:],
                                w1t[:, k, ts(m, P)],
                                xt[:, k, ts(n, NF)],
                                start=(k == 0),
                                stop=(k == KD - 1),
                            )
                        nc.scalar.activation(
                            h1t[:, m, :], ps[:], RELU,
                            bias=biast[:, e * 4 + m : e * 4 + m + 1],
                        )
                    h2t = h2pool.tile([P, K2, NF], F32R)
                    for m in range(M2):
                        ps = ps2pool.tile([P, NF], F32)
                        for k in range(K1):
                            nc.tensor.matmul(
                                ps[:],
                                w2t[:, k, ts(m, P)],
                                h1t[:, k, :],
                                start=(k == 0),
                                stop=(k == K1 - 1),
                            )
                        nc.scalar.activation(
                            h2t[:, m, :], ps[:], RELU,
                            bias=biast[:, 52 + e * 2 + m : 52 + e * 2 + m + 1],
                        )
                    ps3 = ps3pool.tile([16, NF], F32)
                    for k in range(K2):
                        nc.tensor.matmul(
                            ps3[:],
                            w3t[:, e, k, :],
                            h2t[:, k, :],
                            start=(k == 0),
                            stop=(k == K2 - 1),
                        )
                    # Expert e only populates PSUM row e (block-diagonal
                    # lhsT); rows of other experts are zero, so accumulate.
                    nc.vector.tensor_add(
                        outt[:, ts(n, NF)], outt[:, ts(n, NF)], ps3[:]
                    )

            for n in range(NB):
                nc.scalar.activation(
                    outt[:, ts(n, NF)], outt[:, ts(n, NF)], IDENT,
                    bias=biast[:16, 78:79],
                )
            nc.sync.dma_start(outd[:], outt[:])

    nc.compile()
    return nc


def _get_nc():
    if "nc" not in _CACHE:
        _CACHE["nc"] = _build()
    return _CACHE["nc"]


def _prep_in_maps(x, W1, b1, W2, b2, W3, b3):
    x = np.asarray(x, dtype=np.float32)
    W1 = np.asarray(W1, dtype=np.float32)
    b1 = np.asarray(b1, dtype=np.float32)
    W2 = np.asarray(W2, dtype=np.float32)
    b2 = np.asarray(b2, dtype=np.float32)
    W3 = np.asarray(W3, dtype=np.float32)
    b3 = np.asarray(b3, dtype=np.float32)

    xt = np.ascontiguousarray(x.T)  # [D, B]

    in_maps = []
    for c in range(NCORES):
        lo = c * EPC
        hi = min(lo + EPC, E)
        ne = hi - lo

        w1c = np.zeros((EPC, D, H1), np.float32)
        w1c[:ne] = W1[lo:hi]
        w2c = np.zeros((EPC, H1, H2), np.float32)
        w2c[:ne] = W2[lo:hi]

        # Block-diagonal layer-3 weights: expert e's W3 vector occupies
        # column e so its dot product lands in PSUM partition row e.
        w3c = np.zeros((EPC, K2, P, 16), np.float32)
        for le in range(ne):
            w3c[le, :, :, le] = W3[lo + le, :, 0].reshape(K2, P)

        biasc = np.zeros((P, 80), np.float32)
        for le in range(ne):
            biasc[:, le * 4 : le * 4 + 4] = b1[lo + le].reshape(4, P).T
            biasc[:, 52 + le * 2 : 52 + le * 2 + 2] = b2[lo + le].reshape(2, P).T
            biasc[le, 78] = b3[lo + le, 0]

        in_maps.append({
            "xt": xt,
            "w1": w1c,
            "w2": w2c,
            "w3": w3c,
            "bias": biasc,
        })
    return in_maps


def kernel(x, W1, b1, W2, b2, W3, b3):
    nc = _get_nc()
    in_maps = _prep_in_maps(x, W1, b1, W2, b2, W3, b3)
    res = run_bass_kernel_spmd(nc, in_maps, core_ids=list(range(NCORES)))
    per_core = [res.results[c]["out"][:EPC] for c in range(NCORES)]  # [13, B] each
    full = np.concatenate(per_core, axis=0)[:E]  # [100, B]
    return np.ascontiguousarray(full.T[:, None, :]).astype(np.float32)  # [B, 1, E]


# revision 10
# speedup vs baseline: 1.0947x; 1.0947x over previous
"""Ensemble of 100 independent 3-layer MLPs on 8 Trainium2 NeuronCores.

Reference computation (E=100, D=2048, H1=512, H2=256, O=1, B=1024):
    h1  = relu(x @ W1[e] + b1[e])      [B, H1]  per expert
    h2  = relu(h1 @ W2[e] + b2[e])     [B, H2]
    out = h2 @ W3[e] + b3[e]           [B, 1]
    result[b, 0, e] = out[b]           -> [B, 1, E]

Sharding: expert-parallel. E=100 padded to 104 = 8 cores x 13 experts.
Each core gets its 13 experts' weights plus a replicated transposed
input xT; it computes out[e_local, b] and the host concatenates.

On-chip layout: activations are kept feature-major ([feature, batch]),
so every layer is matmul(psum, lhsT=W_tile[K,M], rhs=act[K,N]) with the
contraction on partitions and the output already transposed for the
next layer. The only real transpose (x -> xT) happens on the host.

All matmul operands are float32r: fp32 storage, single-pass FP22
matmul on the PE (same throughput as bf16, ~1e-4 relative error).
Layer 3 (O=1) uses a block-diagonal [128, 16] lhsT per expert so each
expert's dot products land in PSUM partition row e; a DVE add folds
them into the output tile.

xT and W1 live in per-k-tile SBUF tiles so the first expert's matmuls
can chase the initial DMA stream instead of waiting for the full
12.6 MB load (Tile dependencies are per-tile).
"""

import sys

if "/opt/trn_rl_repo" not in sys.path:
    sys.path.insert(0, "/opt/trn_rl_repo")

import numpy as np

import concourse.bass as bass
import concourse.tile as tile
from concourse import bacc, mybir
from concourse.bass import ts
from concourse.bass_utils import run_bass_kernel_spmd

F32 = mybir.dt.float32
F32R = mybir.dt.float32r
RELU = mybir.ActivationFunctionType.Relu
IDENT = mybir.ActivationFunctionType.Identity

E, D, H1, H2, B = 100, 2048, 512, 256, 1024
NCORES = 8
EPC = 13          # experts per core (104 padded)
P = 128
KD = D // P       # 16 k-tiles for layer 1
K1 = H1 // P      # 4 k-tiles for layer 2
K2 = H2 // P      # 2 k-tiles for layer 3
M1 = H1 // P      # 4 m-tiles layer 1
M2 = H2 // P      # 2 m-tiles layer 2
NB = 2            # batch split: 2 x 512
NF = B // NB      # 512

_CACHE = {}


def _build():
    nc = bacc.Bacc("TRN2", target_bir_lowering=False)

    xtd = nc.dram_tensor("xt", [D, B], F32R, kind="ExternalInput")
    w1d = nc.dram_tensor("w1", [EPC, D, H1], F32R, kind="ExternalInput")
    w2d = nc.dram_tensor("w2", [EPC, H1, H2], F32R, kind="ExternalInput")
    w3d = nc.dram_tensor("w3", [P, EPC, K2, 16], F32R, kind="ExternalInput")
    biasd = nc.dram_tensor("bias", [P, 80], F32, kind="ExternalInput")
    outd = nc.dram_tensor("out", [16, B], F32, kind="ExternalOutput")

    with tile.TileContext(nc) as tc:
        with (
            tc.tile_pool(name="const", bufs=1) as cpool,
            tc.tile_pool(name="w1p", bufs=2) as w1pool,
            tc.tile_pool(name="w2p", bufs=1) as w2pool,
            tc.tile_pool(name="h1p", bufs=2) as h1pool,
            tc.tile_pool(name="h2p", bufs=2) as h2pool,
        ):
            biast = cpool.tile([P, 80], F32)
            nc.sync.dma_start(biast[:], biasd[:])
            xts = [
                cpool.tile([P, B], F32R, tag=f"xt_{k}", name=f"xt_{k}")
                for k in range(KD)
            ]
            w3t = cpool.tile([P, EPC, K2, 16], F32R)
            outt = cpool.tile([16, B], F32)
            nc.gpsimd.memset(outt[:], 0.0)

            all_w1ks = {}
            all_w2ts = {}

            def load_expert_weights(e):
                w1ks = []
                for k in range(KD):
                    w1k = w1pool.tile(
                        [P, H1], F32R, tag=f"w1_{k}", name=f"w1_{k}_{e}"
                    )
                    nc.sync.dma_start(w1k[:], w1d[e, ts(k, P), :])
                    w1ks.append(w1k)
                    if e == 0:
                        nc.sync.dma_start(xts[k][:], xtd[ts(k, P), :])
                all_w1ks[e] = w1ks
                w2t = w2pool.tile([P, K1, H2], F32R, tag="w2t", name=f"w2t_{e}")
                nc.sync.dma_start(
                    w2t[:], w2d[e].rearrange("(k p) h -> p k h", p=P)
                )
                all_w2ts[e] = w2t

            # Expert 0's layer 1 is DMA-bound: run it k-outer over all 8
            # (m, n) PSUM banks so every arriving (w1_k, xt_k) pair feeds 8
            # matmuls instead of 4, chasing the initial load. The pool is
            # scoped so its 8 banks free up before the steady-state pools.
            load_expert_weights(0)
            nc.sync.dma_start(w3t[:], w3d[:])
            h1ts0 = {}
            with tc.tile_pool(name="pse0p", bufs=1, space="PSUM") as pse0pool:
                pse = {
                    (m, n): pse0pool.tile(
                        [P, NF], F32,
                        tag=f"pse0_{m}_{n}", name=f"pse0_{m}_{n}",
                    )
                    for m in range(M1)
                    for n in range(NB)
                }
                for k in range(KD):
                    for n in range(NB):
                        for m in range(M1):
                            nc.tensor.matmul(
                                pse[(m, n)][:],
                                all_w1ks[0][k][:, ts(m, P)],
                                xts[k][:, ts(n, NF)],
                                start=(k == 0),
                                stop=(k == KD - 1),
                            )
                for n in range(NB):
                    h1t = h1pool.tile([P, K1, NF], F32R, tag="h1t", name=f"h1t_0_{n}")
                    for m in range(M1):
                        nc.scalar.activation(
                            h1t[:, m, :], pse[(m, n)][:], RELU,
                            bias=biast[:, m : m + 1],
                        )
                    h1ts0[n] = h1t

            with (
                tc.tile_pool(name="ps1p", bufs=3, space="PSUM") as ps1pool,
                tc.tile_pool(name="ps2p", bufs=2, space="PSUM") as ps2pool,
                tc.tile_pool(name="ps3p", bufs=2, space="PSUM") as ps3pool,
            ):
                self_loop_body(
                    nc, tc, cpool, w1pool, w2pool, h1pool, h2pool,
                    ps1pool, ps2pool, ps3pool,
                    xts, w3t, biast, outt, outd,
                    load_expert_weights, all_w1ks, all_w2ts, h1ts0,
                )

    nc.compile()
    return nc


def self_loop_body(
    nc, tc, cpool, w1pool, w2pool, h1pool, h2pool,
    ps1pool, ps2pool, ps3pool,
    xts, w3t, biast, outt, outd,
    load_expert_weights, all_w1ks, all_w2ts, h1ts0,
):
    if True:
        if True:
            for e in range(EPC):
                if e > 0:
                    load_expert_weights(e)
                w1ks = all_w1ks[e]
                w2t = all_w2ts[e]
                for n in range(NB):
                    if e == 0:
                        h1t = h1ts0[n]
                    else:
                        h1t = h1pool.tile(
                            [P, K1, NF], F32R, tag="h1t", name=f"h1t_{e}_{n}"
                        )
                        for m in range(M1):
                            ps = ps1pool.tile([P, NF], F32)
                            for k in range(KD):
                                nc.tensor.matmul(
                                    ps[:],
                                    w1ks[k][:, ts(m, P)],
                                    xts[k][:, ts(n, NF)],
                                    start=(k == 0),
                                    stop=(k == KD - 1),
                                )
                            nc.scalar.activation(
                                h1t[:, m, :], ps[:], RELU,
                                bias=biast[:, e * 4 + m : e * 4 + m + 1],
                            )
                    h2t = h2pool.tile([P, K2, NF], F32R)
                    for m in range(M2):
                        ps = ps2pool.tile([P, NF], F32)
                        for k in range(K1):
                            nc.tensor.matmul(
                                ps[:],
                                w2t[:, k, ts(m, P)],
                                h1t[:, k, :],
                                start=(k == 0),
                                stop=(k == K1 - 1),
                            )
                        nc.scalar.activation(
                            h2t[:, m, :], ps[:], RELU,
                            bias=biast[:, 52 + e * 2 + m : 52 + e * 2 + m + 1],
                        )
                    ps3 = ps3pool.tile([16, NF], F32)
                    for k in range(K2):
                        nc.tensor.matmul(
                            ps3[:],
                            w3t[:, e, k, :],
                            h2t[:, k, :],
                            start=(k == 0),
                            stop=(k == K2 - 1),
                        )
                    # Expert e only populates PSUM row e (block-diagonal
                    # lhsT); rows of other experts are zero, so accumulate.
                    nc.vector.tensor_add(
                        outt[:, ts(n, NF)], outt[:, ts(n, NF)], ps3[:]
                    )

            for n in range(NB):
                nc.scalar.activation(
                    outt[:, ts(n, NF)], outt[:, ts(n, NF)], IDENT,
                    bias=biast[:16, 78:79],
                )
            nc.sync.dma_start(outd[:], outt[:])


def _get_nc():
    if "nc" not in _CACHE:
        _CACHE["nc"] = _build()
    return _CACHE["nc"]


def _prep_in_maps(x, W1, b1, W2, b2, W3, b3):
    x = np.asarray(x, dtype=np.float32)
    W1 = np.asarray(W1, dtype=np.float32)
    b1 = np.asarray(b1, dtype=np.float32)
    W2 = np.asarray(W2, dtype=np.float32)
    b2 = np.asarray(b2, dtype=np.float32)
    W3 = np.asarray(W3, dtype=np.float32)
    b3 = np.asarray(b3, dtype=np.float32)

    xt = np.ascontiguousarray(x.T)  # [D, B]

    in_maps = []
    for c in range(NCORES):
        lo = c * EPC
        hi = min(lo + EPC, E)
        ne = hi - lo

        w1c = np.zeros((EPC, D, H1), np.float32)
        w1c[:ne] = W1[lo:hi]
        w2c = np.zeros((EPC, H1, H2), np.float32)
        w2c[:ne] = W2[lo:hi]

        # Block-diagonal layer-3 weights: expert e's W3 vector occupies
        # column e so its dot product lands in PSUM partition row e.
        w3c = np.zeros((P, EPC, K2, 16), np.float32)
        for le in range(ne):
            w3c[:, le, :, le] = W3[lo + le, :, 0].reshape(K2, P).T

        biasc = np.zeros((P, 80), np.float32)
        for le in range(ne):
            biasc[:, le * 4 : le * 4 + 4] = b1[lo + le].reshape(4, P).T
            biasc[:, 52 + le * 2 : 52 + le * 2 + 2] = b2[lo + le].reshape(2, P).T
            biasc[le, 78] = b3[lo + le, 0]

        in_maps.append({
            "xt": xt,
            "w1": w1c,
            "w2": w2c,
            "w3": w3c,
            "bias": biasc,
        })
    return in_maps


def kernel(x, W1, b1, W2, b2, W3, b3):
    nc = _get_nc()
    in_maps = _prep_in_maps(x, W1, b1, W2, b2, W3, b3)
    res = run_bass_kernel_spmd(nc, in_maps, core_ids=list(range(NCORES)))
    per_core = [res.results[c]["out"][:EPC] for c in range(NCORES)]  # [13, B] each
    full = np.concatenate(per_core, axis=0)[:E]  # [100, B]
    return np.ascontiguousarray(full.T[:, None, :]).astype(np.float32)  # [B, 1, E]


# revision 12
# speedup vs baseline: 1.1095x; 1.0135x over previous
"""Ensemble of 100 independent 3-layer MLPs on 8 Trainium2 NeuronCores.

Reference computation (E=100, D=2048, H1=512, H2=256, O=1, B=1024):
    h1  = relu(x @ W1[e] + b1[e])      [B, H1]  per expert
    h2  = relu(h1 @ W2[e] + b2[e])     [B, H2]
    out = h2 @ W3[e] + b3[e]           [B, 1]
    result[b, 0, e] = out[b]           -> [B, 1, E]

Sharding: expert-parallel. E=100 padded to 104 = 8 cores x 13 experts.
Each core gets its 13 experts' weights plus a replicated transposed
input xT; it computes out[e_local, b] and the host concatenates.

On-chip layout: activations are kept feature-major ([feature, batch]),
so every layer is matmul(psum, lhsT=W_tile[K,M], rhs=act[K,N]) with the
contraction on partitions and the output already transposed for the
next layer. The only real transpose (x -> xT) happens on the host.

All matmul operands are float32r: fp32 storage, single-pass FP22
matmul on the PE (same throughput as bf16, ~1e-4 relative error).
Layer 3 (O=1) uses a block-diagonal [128, 16] lhsT per expert so each
expert's dot products land in PSUM partition row e; a DVE add folds
them into the output tile.

xT and W1 live in per-k-tile SBUF tiles so the first expert's matmuls
can chase the initial DMA stream instead of waiting for the full
12.6 MB load (Tile dependencies are per-tile).
"""

import sys

if "/opt/trn_rl_repo" not in sys.path:
    sys.path.insert(0, "/opt/trn_rl_repo")

import numpy as np

import concourse.bass as bass
import concourse.tile as tile
from concourse import bacc, mybir
from concourse.bass import ts
from concourse.bass_utils import run_bass_kernel_spmd

F32 = mybir.dt.float32
F32R = mybir.dt.float32r
RELU = mybir.ActivationFunctionType.Relu
IDENT = mybir.ActivationFunctionType.Identity

E, D, H1, H2, B = 100, 2048, 512, 256, 1024
NCORES = 8
EPC = 13          # experts per core (104 padded)
P = 128
KD = D // P       # 16 k-tiles for layer 1
K1 = H1 // P      # 4 k-tiles for layer 2
K2 = H2 // P      # 2 k-tiles for layer 3
M1 = H1 // P      # 4 m-tiles layer 1
M2 = H2 // P      # 2 m-tiles layer 2
NB = 2            # batch split: 2 x 512
NF = B // NB      # 512

_CACHE = {}


def _build():
    nc = bacc.Bacc("TRN2", target_bir_lowering=False)

    xtd = nc.dram_tensor("xt", [D, B], F32R, kind="ExternalInput")
    w1d = nc.dram_tensor("w1", [EPC, D, H1], F32R, kind="ExternalInput")
    w2d = nc.dram_tensor("w2", [EPC, H1, H2], F32R, kind="ExternalInput")
    w3d = nc.dram_tensor("w3", [P, EPC, K2, 16], F32R, kind="ExternalInput")
    biasd = nc.dram_tensor("bias", [P, 80], F32, kind="ExternalInput")
    outd = nc.dram_tensor("out", [16, B], F32, kind="ExternalOutput")

    with tile.TileContext(nc) as tc:
        with (
            tc.tile_pool(name="const", bufs=1) as cpool,
            tc.tile_pool(name="w1p", bufs=2) as w1pool,
            tc.tile_pool(name="w2p", bufs=1) as w2pool,
            tc.tile_pool(name="h1p", bufs=2) as h1pool,
            tc.tile_pool(name="h2p", bufs=2) as h2pool,
        ):
            biast = cpool.tile([P, 80], F32)
            nc.sync.dma_start(biast[:], biasd[:])
            xts = {
                (k, n): cpool.tile(
                    [P, NF], F32R, tag=f"xt_{k}_{n}", name=f"xt_{k}_{n}"
                )
                for k in range(KD)
                for n in range(NB)
            }
            w3t = cpool.tile([P, EPC, K2, 16], F32R)
            outt = cpool.tile([16, B], F32)
            nc.gpsimd.memset(outt[:], 0.0)

            all_w1ks = {}
            all_w2ts = {}

            def load_expert_weights(e):
                w1ks = []
                for k in range(KD):
                    w1k = w1pool.tile(
                        [P, H1], F32R, tag=f"w1_{k}", name=f"w1_{k}_{e}"
                    )
                    nc.sync.dma_start(w1k[:], w1d[e, ts(k, P), :])
                    w1ks.append(w1k)
                    if e == 0:
                        for n in range(NB):
                            nc.sync.dma_start(
                                xts[(k, n)][:], xtd[ts(k, P), ts(n, NF)]
                            )
                all_w1ks[e] = w1ks
                w2t = w2pool.tile([P, K1, H2], F32R, tag="w2t", name=f"w2t_{e}")
                nc.sync.dma_start(
                    w2t[:], w2d[e].rearrange("(k p) h -> p k h", p=P)
                )
                all_w2ts[e] = w2t

            # Expert 0's layer 1 is DMA-bound: run it k-outer over all 8
            # (m, n) PSUM banks so every arriving (w1_k, xt_k) pair feeds 8
            # matmuls instead of 4, chasing the initial load. The pool is
            # scoped so its 8 banks free up before the steady-state pools.
            load_expert_weights(0)
            nc.sync.dma_start(w3t[:], w3d[:])
            h1ts0 = {}
            with (
                tc.tile_pool(name="pse0a", bufs=1, space="PSUM") as pse0a,
                tc.tile_pool(name="pse0b", bufs=1, space="PSUM") as pse0b,
            ):
                pools0 = {0: pse0a, 1: pse0b}
                pse = {
                    (m, n): pools0[n].tile(
                        [P, NF], F32,
                        tag=f"pse0_{m}_{n}", name=f"pse0_{m}_{n}",
                    )
                    for m in range(M1)
                    for n in range(NB)
                }
                for k in range(KD):
                    for n in range(NB):
                        for m in range(M1):
                            nc.tensor.matmul(
                                pse[(m, n)][:],
                                all_w1ks[0][k][:, ts(m, P)],
                                xts[(k, n)][:],
                                start=(k == 0),
                                stop=(k == KD - 1),
                            )
                for n in range(NB):
                    h1t = h1pool.tile([P, K1, NF], F32R, tag="h1t", name=f"h1t_0_{n}")
                    for m in range(M1):
                        nc.scalar.activation(
                            h1t[:, m, :], pse[(m, n)][:], RELU,
                            bias=biast[:, m : m + 1],
                        )
                    h1ts0[n] = h1t

            with (
                tc.tile_pool(name="ps1p", bufs=3, space="PSUM") as ps1pool,
                tc.tile_pool(name="ps2p", bufs=2, space="PSUM") as ps2pool,
                tc.tile_pool(name="ps3p", bufs=2, space="PSUM") as ps3pool,
            ):
                self_loop_body(
                    nc, tc, cpool, w1pool, w2pool, h1pool, h2pool,
                    ps1pool, ps2pool, ps3pool,
                    xts, w3t, biast, outt, outd,
                    load_expert_weights, all_w1ks, all_w2ts, h1ts0,
                )

    nc.compile()
    return nc


def self_loop_body(
    nc, tc, cpool, w1pool, w2pool, h1pool, h2pool,
    ps1pool, ps2pool, ps3pool,
    xts, w3t, biast, outt, outd,
    load_expert_weights, all_w1ks, all_w2ts, h1ts0,
):
    if True:
        if True:  # keep original indentation
            for e in range(EPC):
                if e > 0:
                    load_expert_weights(e)
                w1ks = all_w1ks[e]
                w2t = all_w2ts[e]
                for n in range(NB):
                    if e == 0:
                        h1t = h1ts0[n]
                    else:
                        h1t = h1pool.tile(
                            [P, K1, NF], F32R, tag="h1t", name=f"h1t_{e}_{n}"
                        )
                        for m in range(M1):
                            ps = ps1pool.tile([P, NF], F32)
                            for k in range(KD):
                                nc.tensor.matmul(
                                    ps[:],
                                    w1ks[k][:, ts(m, P)],
                                    xts[(k, n)][:],
                                    start=(k == 0),
                                    stop=(k == KD - 1),
                                )
                            nc.scalar.activation(
                                h1t[:, m, :], ps[:], RELU,
                                bias=biast[:, e * 4 + m : e * 4 + m + 1],
                            )
                    h2t = h2pool.tile([P, K2, NF], F32R)
                    for m in range(M2):
                        ps = ps2pool.tile([P, NF], F32)
                        for k in range(K1):
                            nc.tensor.matmul(
                                ps[:],
                                w2t[:, k, ts(m, P)],
                                h1t[:, k, :],
                                start=(k == 0),
                                stop=(k == K1 - 1),
                            )
                        nc.scalar.activation(
                            h2t[:, m, :], ps[:], RELU,
                            bias=biast[:, 52 + e * 2 + m : 52 + e * 2 + m + 1],
                        )
                    ps3 = ps3pool.tile([16, NF], F32)
                    for k in range(K2):
                        nc.tensor.matmul(
                            ps3[:],
                            w3t[:, e, k, :],
                            h2t[:, k, :],
                            start=(k == 0),
                            stop=(k == K2 - 1),
                        )
                    # Expert e only populates PSUM row e (block-diagonal
                    # lhsT); rows of other experts are zero, so accumulate.
                    nc.vector.tensor_add(
                        outt[:, ts(n, NF)], outt[:, ts(n, NF)], ps3[:]
                    )

            for n in range(NB):
                nc.scalar.activation(
                    outt[:, ts(n, NF)], outt[:, ts(n, NF)], IDENT,
                    bias=biast[:16, 78:79],
                )
            nc.sync.dma_start(outd[:], outt[:])


def _get_nc():
    if "nc" not in _CACHE:
        _CACHE["nc"] = _build()
    return _CACHE["nc"]


def _prep_in_maps(x, W1, b1, W2, b2, W3, b3):
    x = np.asarray(x, dtype=np.float32)
    W1 = np.asarray(W1, dtype=np.float32)
    b1 = np.asarray(b1, dtype=np.float32)
    W2 = np.asarray(W2, dtype=np.float32)
    b2 = np.asarray(b2, dtype=np.float32)
    W3 = np.asarray(W3, dtype=np.float32)
    b3 = np.asarray(b3, dtype=np.float32)

    xt = np.ascontiguousarray(x.T)  # [D, B]

    in_maps = []
    for c in range(NCORES):
        lo = c * EPC
        hi = min(lo + EPC, E)
        ne = hi - lo

        w1c = np.zeros((EPC, D, H1), np.float32)
        w1c[:ne] = W1[lo:hi]
        w2c = np.zeros((EPC, H1, H2), np.float32)
        w2c[:ne] = W2[lo:hi]

        # Block-diagonal layer-3 weights: expert e's W3 vector occupies
        # column e so its dot product lands in PSUM partition row e.
        w3c = np.zeros((P, EPC, K2, 16), np.float32)
        for le in range(ne):
            w3c[:, le, :, le] = W3[lo + le, :, 0].reshape(K2, P).T

        biasc = np.zeros((P, 80), np.float32)
        for le in range(ne):
            biasc[:, le * 4 : le * 4 + 4] = b1[lo + le].reshape(4, P).T
            biasc[:, 52 + le * 2 : 52 + le * 2 + 2] = b2[lo + le].reshape(2, P).T
            biasc[le, 78] = b3[lo + le, 0]

        in_maps.append({
            "xt": xt,
            "w1": w1c,
            "w2": w2c,
            "w3": w3c,
            "bias": biasc,
        })
    return in_maps


def kernel(x, W1, b1, W2, b2, W3, b3):
    nc = _get_nc()
    in_maps = _prep_in_maps(x, W1, b1, W2, b2, W3, b3)
    res = run_bass_kernel_spmd(nc, in_maps, core_ids=list(range(NCORES)))
    per_core = [res.results[c]["out"][:EPC] for c in range(NCORES)]  # [13, B] each
    full = np.concatenate(per_core, axis=0)[:E]  # [100, B]
    return np.ascontiguousarray(full.T[:, None, :]).astype(np.float32)  # [B, 1, E]
